# revision 99
# baseline (speedup 1.0000x reference)
"""AutoMTLSuperNet (moe_routing) Trainium2 kernel.

Strategy: batch data-parallel over 8 NeuronCores (2048 samples each, params
replicated). On-chip layout is output-channel-major ([oc, batch]) so every
layer's matmul output is directly the next layer's moving operand. All
matmuls run in bf16 with f32 PSUM accumulation; batch is processed in
chunks of 512 columns.

Host-side prep is parameter-only + input layout: transposes, padding,
folding sigmoid(feat_alpha) into the L0 weight rows, FM contraction
matrices, candidate-softmax weights, merged gate biases. Dense features
ride in the zero-padding rows of xT so the hybrid k-tile loads in one DMA.

Engine balance (measured per-op costs on HW): all candidate-relu branches
run on ACT (relu(w*p + w*b) via the scale port); gate softmaxes are
normalized up front (reciprocal_approx_fast + row-broadcast matmul) so the
expert mix needs no per-n reciprocal multiply; half the expert-mix
broadcasts are consumed directly from f32 PSUM by DVE, half via ACT
psum->bf16 copies; the final domain mix is computed expert-major (PE row
broadcasts of the domain-selected, normalized gate weights) and transposed
once at the end.
"""

import numpy as np
import ml_dtypes

import concourse.bass as bass
import concourse.bacc as bacc
import concourse.mybir as mybir
import concourse.tile as tile
from concourse.bass_utils import run_bass_kernel_spmd

# ---- problem dims (hardcoded per contract) ----
B, F, E, D = 16384, 26, 16, 13
NE, ND, NC = 4, 3, 3
GIN = E * (F + 1) + D            # 445
H, OUT = 256, 128
N_CORES = 8
B_LOC = B // N_CORES             # 2048
NBC = 512                        # batch columns per chunk
NCHUNK = B_LOC // NBC            # 4
KSP = F * E                      # 416 flattened sparse dim
KPAD = 448                       # padded to 4 x (128,128,128,64)
BF16 = mybir.dt.bfloat16
F32 = mybir.dt.float32

AF = mybir.ActivationFunctionType
ALU = mybir.AluOpType


# ---- packed weight bundle layouts (name -> (col_off, rows, cols)) ----
def _mk_layout(blocks):
    out, cur = {}, 0
    for name, rows, cols in blocks:
        out[name] = (cur, rows, cols)
        cur += cols
    return out, cur

KT_ROWS = [128, 128, 128, 64]
# bundle A: phase0-critical gates; bundle R: everything else (Wl0 first so
# it lands before L0b0 needs it). All loaded in the prologue.
_WBA_BLOCKS = (
    [(f'Gs{k}', KT_ROWS[k], 64) for k in range(4)]
    + [(f'Gq{k}', KT_ROWS[k], 64) for k in range(4)]
    + [(f'Wg{k}', KT_ROWS[k], 44) for k in range(4)]
    + [('sel16', 16, 4), ('r16sel', 4, 16), ('oh3', 3, 12), ('sel12', 12, 36)]
)
_WBR_BLOCKS = (
    [(f'Wl0_{k}', 128, 3072) for k in range(4)]
    + [(f'selbc{r}', 16, 128) for r in range(16)]
    + [(f'selbr{r}', 4, 128) for r in range(4)]
    + [(f'Wb1_{n}{k}', 128, 384) for n in range(4) for k in range(2)]
    + [(f'W10_{n}', 128, 768) for n in range(4)]
    + [(f'W11_{n}{k}', 128, 384) for n in range(4) for k in range(2)]
    + [('ident', 128, 128)]
)
_WF32_BLOCKS = [('gbias', 44, 1), ('bl0b0', 128, 24), ('bl0b1', 128, 12),
                ('bl1b0', 128, 24), ('bl1b1', 128, 12), ('wmix', 128, 48)]
WBA_LAYOUT, WBA_COLS = _mk_layout(_WBA_BLOCKS)
WBR_LAYOUT, WBR_COLS = _mk_layout(_WBR_BLOCKS)
WF32_LAYOUT, WF32_COLS = _mk_layout(_WF32_BLOCKS)


def _bf16(x):
    return np.asarray(x, dtype=ml_dtypes.bfloat16)


def _softmax_np(a):
    a = np.asarray(a, dtype=np.float64)
    m = a.max(axis=-1, keepdims=True)
    e = np.exp(a - m)
    return (e / e.sum(axis=-1, keepdims=True)).astype(np.float32)


def prep_shared(inputs):
    """Host prep of all parameter tensors (input-layout + parameter-only math)."""
    f32 = np.float32
    gate_w = 1.0 / (1.0 + np.exp(-inputs['feat_alpha'].astype(np.float64)))  # [NE,F]
    gate_w = gate_w.astype(f32)

    W_l0b0 = inputs['W_l0b0'].astype(f32)   # [NE,NC,GIN,H]
    W_l0b1 = inputs['W_l0b1'].astype(f32)   # [NE,NC,H,OUT]
    W_l1b0 = inputs['W_l1b0'].astype(f32)   # [NE,NC,OUT,H]
    W_l1b1 = inputs['W_l1b1'].astype(f32)   # [NE,NC,H,OUT]

    # candidate softmax weights per mixed-op layer: [4][NE,NC]
    wmix_l = [_softmax_np(inputs[k]) for k in ('a_l0b0', 'a_l0b1', 'a_l1b0', 'a_l1b1')]

    # ---- Wl0: lhsT ktiles [4,128, 3072]; col = n*768 + c*256 + h ----
    Wl0 = np.zeros((4, 128, NE * NC * H), dtype=f32)
    # sparse rows (g-folded): global row i = fe for fe in 0..415
    Wsp = np.zeros((KSP, NE, NC, H), dtype=f32)
    for n in range(NE):
        gvec = np.repeat(gate_w[n], E)                      # [416]
        Wsp[:, n] = W_l0b0[n, :, :KSP, :].transpose(1, 0, 2) * gvec[:, None, None]
    Wsp = Wsp.reshape(KSP, NE * NC * H)
    for kt in range(3):
        Wl0[kt, :, :] = Wsp[kt * 128:(kt + 1) * 128]
    # kt3 layout: [0:32]=sparse rows 384..415, [32:45]=dense, [45:64]=0,
    #             [64:128]=fm rows (64 + n*16 + e)
    Wl0[3, 0:32, :] = Wsp[384:416]
    for d in range(D):
        Wl0[3, 32 + d, :] = W_l0b0[:, :, KSP + E + d, :].reshape(-1)
    for n in range(NE):
        for e in range(E):
            Wl0[3, 64 + n * 16 + e, n * 768:(n + 1) * 768] = \
                W_l0b0[n, :, KSP + e, :].reshape(768)

    # ---- Gs / Gq: [4,128,64]  col = n*16+e ; row = fe (per ktile of xT) ----
    Gs = np.zeros((4, 128, 64), dtype=f32)
    Gq = np.zeros((4, 128, 64), dtype=f32)
    for fe in range(KSP):
        kt, i = divmod(fe, 128)
        f_, e_ = divmod(fe, E)
        for n in range(NE):
            g = gate_w[n, f_]
            Gs[kt, i, n * 16 + e_] = g
            Gq[kt, i, n * 16 + e_] = 0.5 * g * g   # 0.5 pre-folded
    # ---- Wg: [4,128,44]: cols 0-15 g0 (e*4+n), 32-43 g1 (32+d*4+e) ----
    Wg = np.zeros((4, 128, 44), dtype=f32)
    Wg0, Wg1 = inputs['Wg0'].astype(f32), inputs['Wg1'].astype(f32)
    for i in range(KSP):
        kt, r = divmod(i, 128)
        for n in range(NE):
            for e in range(NE):
                Wg[kt, r, e * 4 + n] = Wg0[n, i, e]
        for d in range(ND):
            for e in range(NE):
                Wg[kt, r, 32 + d * 4 + e] = Wg1[d, i, e]
    gbias = np.zeros((44, 1), dtype=f32)
    for n in range(NE):
        for e in range(NE):
            gbias[e * 4 + n, 0] = inputs['bg0'][n, e] + inputs['beta0'][n, e]
    for d in range(ND):
        for e in range(NE):
            gbias[32 + d * 4 + e, 0] = inputs['bg1'][d, e] + inputs['beta1'][d, e]
    # sel16 [16,4]: row e*4+n -> col n
    sel16 = np.zeros((16, 4), dtype=f32)
    for e in range(NE):
        for n in range(NE):
            sel16[e * 4 + n, n] = 1.0
    # r16sel [4,16]: broadcast r0 row n to rows e*4+n
    r16sel = np.zeros((4, 16), dtype=f32)
    for e in range(NE):
        for n in range(NE):
            r16sel[n, e * 4 + n] = 1.0
    # oh3 [3,12]: broadcast domain-onehot row d to rows 4d..4d+3
    oh3 = np.zeros((3, 12), dtype=f32)
    for d in range(ND):
        oh3[d, 4 * d:4 * d + 4] = 1.0
    # sel12 [12,36]: cols 0:4 = all-ones (expert sum); cols 32:36 pick expert e
    sel12 = np.zeros((12, 36), dtype=f32)
    sel12[:, 0:4] = 1.0
    for d in range(ND):
        for e in range(NE):
            sel12[4 * d + e, 32 + e] = 1.0

    # ---- later layer weights ----
    Wb1 = np.zeros((NE, H, NC * OUT), dtype=f32)       # lhsT col = c*128+o
    for n in range(NE):
        Wb1[n] = W_l0b1[n].transpose(1, 0, 2).reshape(H, NC * OUT)
    W10 = np.zeros((NE, OUT, NC * H), dtype=f32)       # col = c*256+h
    for n in range(NE):
        W10[n] = W_l1b0[n].transpose(1, 0, 2).reshape(OUT, NC * H)
    W11 = np.zeros((NE, H, NC * OUT), dtype=f32)
    for n in range(NE):
        W11[n] = W_l1b1[n].transpose(1, 0, 2).reshape(H, NC * OUT)

    # ---- bias column tables (per-partition vectors), w-scaled for relu c=0 ----
    def bias_table(bmat, wl, n_mt):  # bmat [NE,NC,W]; returns [128, NE*NC*n_mt]
        Wd = bmat.shape[-1]
        tbl = np.zeros((128, NE * NC * (Wd // 128)), dtype=f32)
        m = 0
        for n in range(NE):
            for c in range(NC):
                for hh in range(Wd // 128):
                    v = bmat[n, c, hh * 128:(hh + 1) * 128].astype(f32)
                    if c == 0:
                        v = v * wl[n, 0]
                    tbl[:, m] = v
                    m += 1
        return tbl
    bl0b0 = bias_table(inputs['b_l0b0'], wmix_l[0], 2)   # [128,24]
    bl0b1 = bias_table(inputs['b_l0b1'], wmix_l[1], 1)   # [128,12]
    bl1b0 = bias_table(inputs['b_l1b0'], wmix_l[2], 2)   # [128,24]
    bl1b1 = bias_table(inputs['b_l1b1'], wmix_l[3], 1)   # [128,12]

    wmix = np.zeros((128, 48), dtype=f32)
    for li, wl in enumerate(wmix_l):
        for n in range(NE):
            for c in range(NC):
                wmix[:, li * 12 + n * 3 + c] = wl[n, c]

    ident = np.eye(128, dtype=f32)
    ones1 = np.ones((1, 128), dtype=f32)
    # broadcast selectors: selbc[r] = e_r (x) ones128  -> lhsT picks row r of rhs
    selbc = np.zeros((16, 16, 128), dtype=f32)
    for r_ in range(16):
        selbc[r_, r_, :] = 1.0
    selbr = np.zeros((4, 4, 128), dtype=f32)
    for r_ in range(4):
        selbr[r_, r_, :] = 1.0

    # pack everything into a few bundles so the device needs few DMAs
    wba = np.zeros((128, WBA_COLS), dtype=ml_dtypes.bfloat16)
    wbr = np.zeros((128, WBR_COLS), dtype=ml_dtypes.bfloat16)
    wf32 = np.zeros((128, WF32_COLS), dtype=f32)

    def put16(name, arr):
        for tbl, arr_np in ((WBA_LAYOUT, wba), (WBR_LAYOUT, wbr)):
            if name in tbl:
                off, rows, cols = tbl[name]
                arr_np[0:rows, off:off + cols] = _bf16(arr)
                return
        raise KeyError(name)

    def put32(name, arr):
        off, rows, cols = WF32_LAYOUT[name]
        wf32[0:rows, off:off + cols] = arr

    for k in range(4):
        put16(f'Wl0_{k}', Wl0[k])
        put16(f'Gs{k}', Gs[k][:KT_ROWS[k]])
        put16(f'Gq{k}', Gq[k][:KT_ROWS[k]])
        put16(f'Wg{k}', Wg[k][:KT_ROWS[k]])
    put16('sel16', sel16); put16('r16sel', r16sel)
    put16('oh3', oh3); put16('sel12', sel12)
    for r_ in range(16):
        put16(f'selbc{r_}', selbc[r_])
    for r_ in range(4):
        put16(f'selbr{r_}', selbr[r_])
    for n in range(NE):
        for k in range(2):
            put16(f'Wb1_{n}{k}', Wb1[n][k * 128:(k + 1) * 128, :])
            put16(f'W11_{n}{k}', W11[n][k * 128:(k + 1) * 128, :])
        put16(f'W10_{n}', W10[n])
    put16('ident', ident)
    put32('gbias', gbias); put32('bl0b0', bl0b0); put32('bl0b1', bl0b1)
    put32('bl1b0', bl1b0); put32('bl1b1', bl1b1); put32('wmix', wmix)
    return {'wba': wba, 'wbr': wbr, 'wf32': wf32}


def prep_core(inputs, r):
    """Per-core input shards (layout only)."""
    lo, hi = r * B_LOC, (r + 1) * B_LOC
    xs = inputs['sparse_embs'][lo:hi].reshape(B_LOC, KSP)      # [2048,416] f32
    xT = np.zeros((KPAD, B_LOC), dtype=ml_dtypes.bfloat16)
    xT[:KSP] = _bf16(xs.T)
    # dense features ride in the padding rows 416:429 (k-tile 3 rows 32:45)
    xT[KSP:KSP + D] = _bf16(inputs['dense_features'][lo:hi].astype(np.float32).T)
    dom = inputs['domain_ids'][lo:hi].astype(np.int64)
    dom1h = np.zeros((ND, B_LOC), dtype=ml_dtypes.bfloat16)
    for d in range(ND):
        dom1h[d] = (dom == d).astype(np.float32)
    return {'xT': xT, 'dom1h': dom1h}


def build_program(relu_dve=True):
    """relu_dve: move L0b0/L1b0 relu branches to DVE tensor_scalar (max,mult).
    Only valid when b_l0b0/b_l1b0 are all-zero (checked by caller)."""
    nc = bacc.Bacc(trn_type="TRN2", target_bir_lowering=False, debug=False)

    # ---- DRAM I/O ----
    t_xT = nc.dram_tensor('xT', [KPAD, B_LOC], BF16, kind="ExternalInput").ap()
    t_dom1h = nc.dram_tensor('dom1h', [ND, B_LOC], BF16, kind="ExternalInput").ap()
    t_wba = nc.dram_tensor('wba', [128, WBA_COLS], BF16, kind="ExternalInput").ap()
    t_wbr = nc.dram_tensor('wbr', [128, WBR_COLS], BF16, kind="ExternalInput").ap()
    t_wf32 = nc.dram_tensor('wf32', [128, WF32_COLS], F32, kind="ExternalInput").ap()
    t_out = nc.dram_tensor('out', [B_LOC, OUT], F32, kind="ExternalOutput").ap()

    KT_ROWS = [128, 128, 128, 64]   # xT sbuf k-tiling
    K3 = 128

    with tile.TileContext(nc) as tc:
        with (
            tc.tile_pool(name="wpool", bufs=1) as wpool,
            tc.tile_pool(name="xpool", bufs=4) as xpool,
            tc.tile_pool(name="apool", bufs=2) as apool,
            tc.tile_pool(name="hpool", bufs=2) as hpool,
            tc.tile_pool(name="bcpool", bufs=4) as bcpool,
            tc.tile_pool(name="spool", bufs=4) as spool,
            tc.tile_pool(name="opool", bufs=2) as opool,
            tc.tile_pool(name="ps_mm", bufs=3, space="PSUM") as ps_mm,
            tc.tile_pool(name="ps_smlt", bufs=2, space="PSUM") as ps_smlt,
            tc.tile_pool(name="ps_bc", bufs=3, space="PSUM") as ps_bc,
        ):
            # ---- prologue: resident weights/constants ----
            # bulk weight loads, small phase0-critical bundle first
            wbaT = wpool.tile([128, WBA_COLS], BF16, tag="wba", name="wba")
            nc.sync.dma_start(wbaT[:], t_wba)
            wfT = wpool.tile([128, WF32_COLS], F32, tag="wf32", name="wf32")
            nc.sync.dma_start(wfT[:], t_wf32)
            wbrT = wpool.tile([128, WBR_COLS], BF16, tag="wbr", name="wbr")
            nc.sync.dma_start(wbrT[:], t_wbr)

            def S16(name):
                for tbl, tile_ in ((WBA_LAYOUT, wbaT), (WBR_LAYOUT, wbrT)):
                    if name in tbl:
                        off, rows, cols = tbl[name]
                        return tile_[0:rows, off:off + cols]
                raise KeyError(name)

            def S32(name):
                off, rows, cols = WF32_LAYOUT[name]
                return wfT[0:rows, off:off + cols]

            sWl0 = [S16(f'Wl0_{kt}') for kt in range(4)]
            sGs = [S16(f'Gs{kt}') for kt in range(4)]
            sGq = [S16(f'Gq{kt}') for kt in range(4)]
            sWg = [S16(f'Wg{kt}') for kt in range(4)]
            sSel = S16('sel16')
            sR16 = S16('r16sel')
            sOh3 = S16('oh3')
            sSel12 = S16('sel12')
            sWb1 = [[S16(f'Wb1_{n}{kt}') for kt in range(2)] for n in range(NE)]
            sW10 = [S16(f'W10_{n}') for n in range(NE)]
            sW11 = [[S16(f'W11_{n}{kt}') for kt in range(2)] for n in range(NE)]
            sGb = S32('gbias')
            sB00 = S32('bl0b0')
            sB01 = S32('bl0b1')
            sB10 = S32('bl1b0')
            sB11 = S32('bl1b1')
            sWmix = S32('wmix')
            sId = S16('ident')
            sSelBc = [S16(f'selbc{r}') for r in range(16)]
            sSelBr = [S16(f'selbr{r}') for r in range(4)]

            # per-chunk state carried between phases
            xk = [None] * NCHUNK
            hyb = [None] * NCHUNK
            e0bf = [None] * NCHUNK
            e1bf = [None] * NCHUNK
            e0n = [None] * NCHUNK
            wn = [None] * NCHUNK
            hA = [None] * NCHUNK
            hB = [None] * NCHUNK
            mixed = [None] * NCHUNK
            hC = [None] * NCHUNK
            h2 = [None] * NCHUNK

            def mixed_op_tail(p, out_t, c, bcol, wcol, relu_on_dve, tmp_tag):
                """candidate-mix tail for one [128,NBC] branch psum tile."""
                if c == 0:
                    if relu_on_dve:
                        nc.vector.tensor_scalar(out_t[:], p[:], 0.0, wcol,
                                                ALU.max, ALU.mult)
                    else:
                        nc.scalar.activation(out_t[:], p[:], AF.Relu,
                                             bias=bcol, scale=wcol)
                else:
                    fn = AF.Gelu_apprx_tanh if c == 1 else AF.Tanh
                    tmp = apool.tile([128, NBC], BF16, tag=tmp_tag,
                                     name=f"t{tmp_tag}_{next(uid)}")
                    nc.scalar.activation(tmp[:], p[:], fn, bias=bcol)
                    tw = apool.tile([128, NBC], BF16, tag="tw" + tmp_tag,
                                    name=f"w{tmp_tag}_{next(uid)}")
                    nc.vector.tensor_scalar(tw[:], tmp[:], wcol, None, ALU.mult)
                    nc.vector.tensor_tensor(out_t[:], out_t[:], tw[:], ALU.add)

            import itertools
            uid = itertools.count()

            # ============ P0: loads, squares, fm, gates, softmax prep ============
            def phase0(ch):
                cc = ch * NBC
                xk[ch] = []
                for kt in range(4):
                    rows = 128 if kt == 3 else KT_ROWS[kt]
                    t = xpool.tile([rows, NBC], BF16, tag=f"x{kt}", name=f"x{kt}_{ch}")
                    nc.sync.dma_start(t[0:KT_ROWS[kt], :],
                                      t_xT[kt * 128: kt * 128 + KT_ROWS[kt], cc:cc + NBC])
                    xk[ch].append(t)
                # dense features arrive inside xT rows 416:429; rows 64:128 of
                # the kt3 tile become the fm features below.
                hyb[ch] = xk[ch][3]
                oh = xpool.tile([ND, NBC], BF16, tag="oh", name=f"oh_{ch}")
                nc.sync.dma_start(oh[:], t_dom1h[:, cc:cc + NBC])

                xq = []
                for kt in range(4):
                    t = xpool.tile([KT_ROWS[kt], NBC], BF16, tag=f"xq{kt}", name=f"xq{kt}_{ch}", bufs=2)
                    src = xk[ch][kt][0:KT_ROWS[kt], :]
                    nc.vector.tensor_tensor(t[:], src, src, ALU.mult)
                    xq.append(t)

                sq_ps = ps_smlt.tile([128, NBC], F32, tag="smlt", name=f"sq_{ch}")
                for kt in range(4):
                    nc.tensor.matmul(sq_ps[0:64, :], sGs[kt][:],
                                     xk[ch][kt][0:KT_ROWS[kt], :],
                                     start=(kt == 0), stop=(kt == 3))
                for kt in range(4):
                    nc.tensor.matmul(sq_ps[64:128, :], sGq[kt][:], xq[kt][:],
                                     start=(kt == 0), stop=(kt == 3))
                ssq = spool.tile([64, NBC], F32, tag="ssq", name=f"ssq_{ch}", bufs=2)
                nc.scalar.activation(ssq[:], sq_ps[0:64, :], AF.Square,
                                     scale=float(np.sqrt(0.5)))
                nc.vector.tensor_tensor(hyb[ch][64:128, :], ssq[:], sq_ps[64:128, :],
                                        ALU.subtract)

                g_ps = ps_smlt.tile([44, NBC], F32, tag="smlt", name=f"g_{ch}")
                for kt in range(4):
                    nc.tensor.matmul(g_ps[:], sWg[kt][:],
                                     xk[ch][kt][0:KT_ROWS[kt], :],
                                     start=(kt == 0), stop=(kt == 3))
                gexp = spool.tile([44, NBC], F32, tag="gexp", name=f"gexp_{ch}", bufs=2)
                nc.scalar.activation(gexp[:], g_ps[:], AF.Exp, bias=sGb[:, 0:1])
                e0 = spool.tile([16, NBC], BF16, tag="e0bf", name=f"e0_{ch}", bufs=2)
                nc.vector.tensor_copy(e0[:], gexp[0:16, :])
                e0bf[ch] = e0
                e1 = spool.tile([12, NBC], BF16, tag="e1bf", name=f"e1_{ch}")
                nc.vector.tensor_copy(e1[:], gexp[32:44, :])
                e1bf[ch] = e1
                # layer-0 gate softmax normalize: e0n = e0 * bcast16(1/rowsum)
                s_ps = ps_smlt.tile([4, NBC], F32, tag="smlt", name=f"s0_{ch}")
                nc.tensor.matmul(s_ps[:], sSel[:], e0[:], start=True, stop=True)
                rf = spool.tile([4, NBC], F32, tag="r0f", name=f"r0f_{ch}", bufs=2)
                nc.vector.reciprocal_approx_fast(rf[:], s_ps[:])
                r = spool.tile([4, NBC], BF16, tag="r0", name=f"r0_{ch}", bufs=2)
                nc.vector.tensor_scalar(r[:], rf[:], 1.0, None, ALU.mult)
                r16_ps = ps_smlt.tile([16, NBC], F32, tag="smlt", name=f"r16_{ch}")
                nc.tensor.matmul(r16_ps[:], sR16[:], r[:], start=True, stop=True)
                en = spool.tile([16, NBC], BF16, tag="e0n", name=f"e0n_{ch}")
                nc.vector.tensor_tensor(en[:], e0[:], r16_ps[:], ALU.mult)
                e0n[ch] = en
                # domain gate weights: mask by onehot, expert-sum + select, norm
                ohb_ps = ps_smlt.tile([12, NBC], F32, tag="smlt", name=f"ohb_{ch}")
                nc.tensor.matmul(ohb_ps[:], sOh3[:], oh[:], start=True, stop=True)
                ws12 = spool.tile([12, NBC], BF16, tag="ws", name=f"ws_{ch}", bufs=2)
                nc.vector.tensor_tensor(ws12[:], e1[:], ohb_ps[:], ALU.mult)
                sw_ps = ps_smlt.tile([36, NBC], F32, tag="smlt", name=f"sw_{ch}")
                nc.tensor.matmul(sw_ps[:], sSel12[:], ws12[:], start=True, stop=True)
                rw = spool.tile([4, NBC], F32, tag="rw", name=f"rw_{ch}", bufs=2)
                nc.vector.reciprocal_approx_fast(rw[:], sw_ps[0:4, :])
                wnt = spool.tile([4, NBC], BF16, tag="wn", name=f"wn_{ch}")
                nc.vector.tensor_tensor(wnt[:], sw_ps[32:36, :], rw[:], ALU.mult)
                wn[ch] = wnt

            # ============ P1: L0b0 + mixA -> hA ; L0b1 + mixB -> hB ============
            def phase1(ch):
                hA[ch] = {}
                for n in range(NE):
                    for hh in range(2):
                        hA[ch][(n, hh)] = hpool.tile([128, NBC], BF16, tag=f"hA{n}{hh}",
                                                     name=f"hA{n}{hh}_{ch}")
                    for c in range(NC):
                        for hh in range(2):
                            m = n * 6 + c * 2 + hh
                            p = ps_mm.tile([128, NBC], F32, tag="pmm", name=f"pA{m}_{ch}")
                            for kt in range(3):
                                nc.tensor.matmul(p[:], sWl0[kt][:, m * 128:(m + 1) * 128],
                                                 xk[ch][kt][:], start=(kt == 0), stop=False)
                            nc.tensor.matmul(p[:], sWl0[3][0:K3, m * 128:(m + 1) * 128],
                                             hyb[ch][:], start=False, stop=True)
                            mixed_op_tail(p, hA[ch][(n, hh)], c, sB00[:, m:m + 1],
                                          sWmix[:, n * 3 + c: n * 3 + c + 1],
                                          False, f"A{hh}")
                hB[ch] = {}
                for n in range(NE):
                    hb = hpool.tile([128, NBC], BF16, tag=f"hB{n}", name=f"hB{n}_{ch}")
                    hB[ch][n] = hb
                    for c in range(NC):
                        p = ps_mm.tile([128, NBC], F32, tag="pmm", name=f"pB{n}{c}_{ch}")
                        for kt in range(2):
                            nc.tensor.matmul(p[:], sWb1[n][kt][:, c * 128:(c + 1) * 128],
                                             hA[ch][(n, kt)][:], start=(kt == 0), stop=(kt == 1))
                        m = n * 3 + c
                        mixed_op_tail(p, hb, c, sB01[:, m:m + 1],
                                      sWmix[:, 12 + m: 12 + m + 1], False, "B")

            # ============ P2: expert mixing 0 ============
            def phase2(ch):
                # broadcast rows of the NORMALIZED gate weights; no rp multiply
                mixed[ch] = {}
                for n in range(NE):
                    bcb = []
                    for e in range(NE):
                        bp = ps_bc.tile([128, NBC], F32, tag="bcp", name=f"bcp{n}{e}_{ch}")
                        nc.tensor.matmul(bp[:], sSelBc[e * 4 + n][:], e0n[ch][:],
                                         start=True, stop=True)
                        if e % 2 == 0:
                            # ACT copies psum->sbuf bf16; DVE multiplies from sbuf
                            bb = bcpool.tile([128, NBC], BF16, tag="bcb",
                                             name=f"bcb{n}{e}_{ch}")
                            nc.scalar.copy(bb[:], bp[:])
                            bcb.append(bb)
                        else:
                            bcb.append(bp)
                    mx = hpool.tile([128, NBC], BF16, tag=f"mix{n}", name=f"mix{n}_{ch}")
                    t0 = bcpool.tile([128, NBC], BF16, tag="mixacc", name=f"acc{n}_{ch}")
                    nc.vector.tensor_tensor(t0[:], hB[ch][0][:], bcb[0][:], ALU.mult)
                    for e in range(1, NE):
                        t2 = bcpool.tile([128, NBC], BF16, tag="mixt", name=f"mixt{n}{e}_{ch}")
                        nc.vector.tensor_tensor(t2[:], hB[ch][e][:], bcb[e][:], ALU.mult)
                        if e < NE - 1:
                            nc.vector.tensor_tensor(t0[:], t0[:], t2[:], ALU.add)
                        else:
                            nc.vector.tensor_tensor(mx[:], t0[:], t2[:], ALU.add)
                    mixed[ch][n] = mx

            # ============ P3: L1b0 + mixC -> hC ; L1b1 + mixD -> h2 ============
            def phase3(ch):
                hC[ch] = {}
                for n in range(NE):
                    for hh in range(2):
                        hC[ch][(n, hh)] = hpool.tile([128, NBC], BF16, tag=f"hC{n}{hh}",
                                                     name=f"hC{n}{hh}_{ch}")
                    for c in range(NC):
                        for hh in range(2):
                            m = n * 6 + c * 2 + hh
                            mt = c * 2 + hh
                            p = ps_mm.tile([128, NBC], F32, tag="pmm", name=f"pC{m}_{ch}")
                            nc.tensor.matmul(p[:], sW10[n][:, mt * 128:(mt + 1) * 128],
                                             mixed[ch][n][:], start=True, stop=True)
                            mixed_op_tail(p, hC[ch][(n, hh)], c, sB10[:, m:m + 1],
                                          sWmix[:, 24 + n * 3 + c: 24 + n * 3 + c + 1],
                                          False, f"C{hh}")
                h2[ch] = {}
                for n in range(NE):
                    hb = hpool.tile([128, NBC], BF16, tag=f"h2{n}", name=f"h2{n}_{ch}")
                    h2[ch][n] = hb
                    for c in range(NC):
                        p = ps_mm.tile([128, NBC], F32, tag="pmm", name=f"pD{n}{c}_{ch}")
                        for kt in range(2):
                            nc.tensor.matmul(p[:], sW11[n][kt][:, c * 128:(c + 1) * 128],
                                             hC[ch][(n, kt)][:], start=(kt == 0), stop=(kt == 1))
                        m = n * 3 + c
                        mixed_op_tail(p, hb, c, sB11[:, m:m + 1],
                                      sWmix[:, 36 + m: 36 + m + 1], False, "Dx")

            # ============ P4: domain mix (expert-major) + transpose + out ============
            def phase4(ch):
                cc = ch * NBC
                em = opool.tile([128, NBC], BF16, tag="em", name=f"em_{ch}", bufs=1)
                wb = []
                for e in range(2):
                    bp = ps_bc.tile([128, NBC], F32, tag="bcp", name=f"wb{e}_{ch}")
                    nc.tensor.matmul(bp[:], sSelBr[e][:], wn[ch][:],
                                     start=True, stop=True)
                    wb.append(bp)
                nc.vector.tensor_tensor(em[:], h2[ch][0][:], wb[0][:], ALU.mult)
                for e in range(1, NE):
                    if e + 1 < NE:
                        bp = ps_bc.tile([128, NBC], F32, tag="bcp",
                                        name=f"wb{e + 1}_{ch}")
                        nc.tensor.matmul(bp[:], sSelBr[e + 1][:], wn[ch][:],
                                         start=True, stop=True)
                        wb.append(bp)
                    t2 = opool.tile([128, NBC], BF16, tag="emt", name=f"emt{e}_{ch}", bufs=1)
                    nc.vector.tensor_tensor(t2[:], h2[ch][e][:], wb[e][:], ALU.mult)
                    nc.vector.tensor_tensor(em[:], em[:], t2[:], ALU.add)
                tp = ps_smlt.tile([128, NBC], BF16, tag="smlt", name=f"otp_{ch}")
                for bt in range(4):
                    nc.tensor.transpose(tp[:, bt * 128:(bt + 1) * 128],
                                        em[:, bt * 128:(bt + 1) * 128], sId[:])
                ote = opool.tile([128, NBC], F32, tag="ote", name=f"ote_{ch}", bufs=1)
                nc.scalar.copy(ote[:], tp[:])
                for bt in range(4):
                    nc.sync.dma_start(t_out[cc + bt * 128: cc + (bt + 1) * 128, :],
                                      ote[:, bt * 128:(bt + 1) * 128])

            # ---- emission schedule: P0 all, then rounds with one-chunk lag ----
            for ch in range(NCHUNK):
                phase0(ch)
            for ch in range(NCHUNK):
                phase1(ch)
                if ch > 0:
                    phase2(ch - 1)
                    phase3(ch - 1)
                    phase4(ch - 1)
            phase2(NCHUNK - 1)
            phase3(NCHUNK - 1)
            phase4(NCHUNK - 1)
    nc.compile()
    return nc


_CACHE = {}


def kernel(**inputs):
    shared = prep_shared(inputs)
    in_maps = []
    for r in range(N_CORES):
        m = dict(shared)
        m.update(prep_core(inputs, r))
        in_maps.append(m)
    relu_dve = (np.abs(inputs['b_l0b0']).max() == 0.0
                and np.abs(inputs['b_l1b0']).max() == 0.0)
    key = ('nc', bool(relu_dve))
    if key not in _CACHE:
        _CACHE[key] = build_program(relu_dve=relu_dve)
        _CACHE['nc'] = _CACHE[key]
    nc = _CACHE[key]
    res = run_bass_kernel_spmd(nc, in_maps, core_ids=list(range(N_CORES)))
    out = np.concatenate([res.results[r]['out'] for r in range(N_CORES)], axis=0)
    return out.astype(np.float32)



# revision 100
# speedup vs baseline: 1.0136x; 1.0136x over previous
"""AutoMTLSuperNet (moe_routing) Trainium2 kernel.

Strategy: batch data-parallel over 8 NeuronCores (2048 samples each, params
replicated). On-chip layout is output-channel-major ([oc, batch]) so every
layer's matmul output is directly the next layer's moving operand. All
matmuls run in bf16 with f32 PSUM accumulation; batch is processed in
chunks of 512 columns.

Host-side prep is parameter-only + input layout: transposes, padding,
folding sigmoid(feat_alpha) into the L0 weight rows, FM contraction
matrices, candidate-softmax weights, merged gate biases. Dense features
ride in the zero-padding rows of xT so the hybrid k-tile loads in one DMA.

Engine balance (measured per-op costs on HW): all candidate-relu branches
run on ACT (relu(w*p + w*b) via the scale port); gate softmaxes are
normalized up front (reciprocal_approx_fast + row-broadcast matmul) so the
expert mix needs no per-n reciprocal multiply; half the expert-mix
broadcasts are consumed directly from f32 PSUM by DVE, half via ACT
psum->bf16 copies; the final domain mix is computed expert-major (PE row
broadcasts of the domain-selected, normalized gate weights) and transposed
once at the end.
"""

import numpy as np
import ml_dtypes

import concourse.bass as bass
import concourse.bacc as bacc
import concourse.mybir as mybir
import concourse.tile as tile
from concourse.bass_utils import run_bass_kernel_spmd

# ---- problem dims (hardcoded per contract) ----
B, F, E, D = 16384, 26, 16, 13
NE, ND, NC = 4, 3, 3
GIN = E * (F + 1) + D            # 445
H, OUT = 256, 128
N_CORES = 8
B_LOC = B // N_CORES             # 2048
NBC = 512                        # batch columns per chunk
NCHUNK = B_LOC // NBC            # 4
KSP = F * E                      # 416 flattened sparse dim
KPAD = 448                       # padded to 4 x (128,128,128,64)
BF16 = mybir.dt.bfloat16
F32 = mybir.dt.float32

AF = mybir.ActivationFunctionType
ALU = mybir.AluOpType


# ---- packed weight bundle layouts (name -> (col_off, rows, cols)) ----
def _mk_layout(blocks):
    out, cur = {}, 0
    for name, rows, cols in blocks:
        out[name] = (cur, rows, cols)
        cur += cols
    return out, cur

KT_ROWS = [128, 128, 128, 64]
_WB16_BLOCKS = (
    [(f'Wl0_{k}', 128, 3072) for k in range(4)]
    + [(f'Gs{k}', KT_ROWS[k], 64) for k in range(4)]
    + [(f'Gq{k}', KT_ROWS[k], 64) for k in range(4)]
    + [(f'Wg{k}', KT_ROWS[k], 44) for k in range(4)]
    + [('sel16', 16, 4), ('r16sel', 4, 16), ('oh3', 3, 12), ('sel12', 12, 36)]
    + [(f'selbc{r}', 16, 128) for r in range(16)]
    + [(f'selbr{r}', 4, 128) for r in range(4)]
    + [(f'Wb1_{n}{k}', 128, 384) for n in range(4) for k in range(2)]
    + [(f'W10_{n}', 128, 768) for n in range(4)]
    + [(f'W11_{n}{k}', 128, 384) for n in range(4) for k in range(2)]
    + [('ident', 128, 128)]
)
_WF32_BLOCKS = [('gbias', 44, 1), ('bl0b0', 128, 24), ('bl0b1', 128, 12),
                ('bl1b0', 128, 24), ('bl1b1', 128, 12), ('wmix', 128, 48)]
WB16_LAYOUT, WB16_COLS = _mk_layout(_WB16_BLOCKS)
WF32_LAYOUT, WF32_COLS = _mk_layout(_WF32_BLOCKS)


def _bf16(x):
    return np.asarray(x, dtype=ml_dtypes.bfloat16)


def _softmax_np(a):
    a = np.asarray(a, dtype=np.float64)
    m = a.max(axis=-1, keepdims=True)
    e = np.exp(a - m)
    return (e / e.sum(axis=-1, keepdims=True)).astype(np.float32)


def prep_shared(inputs):
    """Host prep of all parameter tensors (input-layout + parameter-only math)."""
    f32 = np.float32
    gate_w = 1.0 / (1.0 + np.exp(-inputs['feat_alpha'].astype(np.float64)))  # [NE,F]
    gate_w = gate_w.astype(f32)

    W_l0b0 = inputs['W_l0b0'].astype(f32)   # [NE,NC,GIN,H]
    W_l0b1 = inputs['W_l0b1'].astype(f32)   # [NE,NC,H,OUT]
    W_l1b0 = inputs['W_l1b0'].astype(f32)   # [NE,NC,OUT,H]
    W_l1b1 = inputs['W_l1b1'].astype(f32)   # [NE,NC,H,OUT]

    # candidate softmax weights per mixed-op layer: [4][NE,NC]
    wmix_l = [_softmax_np(inputs[k]) for k in ('a_l0b0', 'a_l0b1', 'a_l1b0', 'a_l1b1')]

    # ---- Wl0: lhsT ktiles [4,128, 3072]; col = n*768 + c*256 + h ----
    Wl0 = np.zeros((4, 128, NE * NC * H), dtype=f32)
    # sparse rows (g-folded): global row i = fe for fe in 0..415
    Wsp = np.zeros((KSP, NE, NC, H), dtype=f32)
    for n in range(NE):
        gvec = np.repeat(gate_w[n], E)                      # [416]
        Wsp[:, n] = W_l0b0[n, :, :KSP, :].transpose(1, 0, 2) * gvec[:, None, None]
    Wsp = Wsp.reshape(KSP, NE * NC * H)
    for kt in range(3):
        Wl0[kt, :, :] = Wsp[kt * 128:(kt + 1) * 128]
    # kt3 layout: [0:32]=sparse rows 384..415, [32:45]=dense, [45:64]=0,
    #             [64:128]=fm rows (64 + n*16 + e)
    Wl0[3, 0:32, :] = Wsp[384:416]
    for d in range(D):
        Wl0[3, 32 + d, :] = W_l0b0[:, :, KSP + E + d, :].reshape(-1)
    for n in range(NE):
        for e in range(E):
            Wl0[3, 64 + n * 16 + e, n * 768:(n + 1) * 768] = \
                W_l0b0[n, :, KSP + e, :].reshape(768)

    # ---- Gs / Gq: [4,128,64]  col = n*16+e ; row = fe (per ktile of xT) ----
    Gs = np.zeros((4, 128, 64), dtype=f32)
    Gq = np.zeros((4, 128, 64), dtype=f32)
    for fe in range(KSP):
        kt, i = divmod(fe, 128)
        f_, e_ = divmod(fe, E)
        for n in range(NE):
            g = gate_w[n, f_]
            Gs[kt, i, n * 16 + e_] = g
            Gq[kt, i, n * 16 + e_] = 0.5 * g * g   # 0.5 pre-folded
    # ---- Wg: [4,128,44]: cols 0-15 g0 (e*4+n), 32-43 g1 (32+d*4+e) ----
    Wg = np.zeros((4, 128, 44), dtype=f32)
    Wg0, Wg1 = inputs['Wg0'].astype(f32), inputs['Wg1'].astype(f32)
    for i in range(KSP):
        kt, r = divmod(i, 128)
        for n in range(NE):
            for e in range(NE):
                Wg[kt, r, e * 4 + n] = Wg0[n, i, e]
        for d in range(ND):
            for e in range(NE):
                Wg[kt, r, 32 + d * 4 + e] = Wg1[d, i, e]
    gbias = np.zeros((44, 1), dtype=f32)
    for n in range(NE):
        for e in range(NE):
            gbias[e * 4 + n, 0] = inputs['bg0'][n, e] + inputs['beta0'][n, e]
    for d in range(ND):
        for e in range(NE):
            gbias[32 + d * 4 + e, 0] = inputs['bg1'][d, e] + inputs['beta1'][d, e]
    # sel16 [16,4]: row e*4+n -> col n
    sel16 = np.zeros((16, 4), dtype=f32)
    for e in range(NE):
        for n in range(NE):
            sel16[e * 4 + n, n] = 1.0
    # r16sel [4,16]: broadcast r0 row n to rows e*4+n
    r16sel = np.zeros((4, 16), dtype=f32)
    for e in range(NE):
        for n in range(NE):
            r16sel[n, e * 4 + n] = 1.0
    # oh3 [3,12]: broadcast domain-onehot row d to rows 4d..4d+3
    oh3 = np.zeros((3, 12), dtype=f32)
    for d in range(ND):
        oh3[d, 4 * d:4 * d + 4] = 1.0
    # sel12 [12,36]: cols 0:4 = all-ones (expert sum); cols 32:36 pick expert e
    sel12 = np.zeros((12, 36), dtype=f32)
    sel12[:, 0:4] = 1.0
    for d in range(ND):
        for e in range(NE):
            sel12[4 * d + e, 32 + e] = 1.0

    # ---- later layer weights ----
    Wb1 = np.zeros((NE, H, NC * OUT), dtype=f32)       # lhsT col = c*128+o
    for n in range(NE):
        Wb1[n] = W_l0b1[n].transpose(1, 0, 2).reshape(H, NC * OUT)
    W10 = np.zeros((NE, OUT, NC * H), dtype=f32)       # col = c*256+h
    for n in range(NE):
        W10[n] = W_l1b0[n].transpose(1, 0, 2).reshape(OUT, NC * H)
    W11 = np.zeros((NE, H, NC * OUT), dtype=f32)
    for n in range(NE):
        W11[n] = W_l1b1[n].transpose(1, 0, 2).reshape(H, NC * OUT)

    # ---- bias column tables (per-partition vectors), w-scaled for relu c=0 ----
    def bias_table(bmat, wl, n_mt):  # bmat [NE,NC,W]; returns [128, NE*NC*n_mt]
        Wd = bmat.shape[-1]
        tbl = np.zeros((128, NE * NC * (Wd // 128)), dtype=f32)
        m = 0
        for n in range(NE):
            for c in range(NC):
                for hh in range(Wd // 128):
                    v = bmat[n, c, hh * 128:(hh + 1) * 128].astype(f32)
                    if c == 0:
                        v = v * wl[n, 0]
                    tbl[:, m] = v
                    m += 1
        return tbl
    bl0b0 = bias_table(inputs['b_l0b0'], wmix_l[0], 2)   # [128,24]
    bl0b1 = bias_table(inputs['b_l0b1'], wmix_l[1], 1)   # [128,12]
    bl1b0 = bias_table(inputs['b_l1b0'], wmix_l[2], 2)   # [128,24]
    bl1b1 = bias_table(inputs['b_l1b1'], wmix_l[3], 1)   # [128,12]

    wmix = np.zeros((128, 48), dtype=f32)
    for li, wl in enumerate(wmix_l):
        for n in range(NE):
            for c in range(NC):
                wmix[:, li * 12 + n * 3 + c] = wl[n, c]

    ident = np.eye(128, dtype=f32)
    ones1 = np.ones((1, 128), dtype=f32)
    # broadcast selectors: selbc[r] = e_r (x) ones128  -> lhsT picks row r of rhs
    selbc = np.zeros((16, 16, 128), dtype=f32)
    for r_ in range(16):
        selbc[r_, r_, :] = 1.0
    selbr = np.zeros((4, 4, 128), dtype=f32)
    for r_ in range(4):
        selbr[r_, r_, :] = 1.0

    # pack everything into two bundles so the device needs just two DMAs
    wb16 = np.zeros((128, WB16_COLS), dtype=ml_dtypes.bfloat16)
    wf32 = np.zeros((128, WF32_COLS), dtype=f32)

    def put16(name, arr):
        off, rows, cols = WB16_LAYOUT[name]
        wb16[0:rows, off:off + cols] = _bf16(arr)

    def put32(name, arr):
        off, rows, cols = WF32_LAYOUT[name]
        wf32[0:rows, off:off + cols] = arr

    for k in range(4):
        put16(f'Wl0_{k}', Wl0[k])
        put16(f'Gs{k}', Gs[k][:KT_ROWS[k]])
        put16(f'Gq{k}', Gq[k][:KT_ROWS[k]])
        put16(f'Wg{k}', Wg[k][:KT_ROWS[k]])
    put16('sel16', sel16); put16('r16sel', r16sel)
    put16('oh3', oh3); put16('sel12', sel12)
    for r_ in range(16):
        put16(f'selbc{r_}', selbc[r_])
    for r_ in range(4):
        put16(f'selbr{r_}', selbr[r_])
    for n in range(NE):
        for k in range(2):
            put16(f'Wb1_{n}{k}', Wb1[n][k * 128:(k + 1) * 128, :])
            put16(f'W11_{n}{k}', W11[n][k * 128:(k + 1) * 128, :])
        put16(f'W10_{n}', W10[n])
    put16('ident', ident)
    put32('gbias', gbias); put32('bl0b0', bl0b0); put32('bl0b1', bl0b1)
    put32('bl1b0', bl1b0); put32('bl1b1', bl1b1); put32('wmix', wmix)
    return {'wb16': wb16, 'wf32': wf32}


def prep_core(inputs, r):
    """Per-core input shards (layout only)."""
    lo, hi = r * B_LOC, (r + 1) * B_LOC
    xs = inputs['sparse_embs'][lo:hi].reshape(B_LOC, KSP)      # [2048,416] f32
    xT = np.zeros((KPAD, B_LOC), dtype=ml_dtypes.bfloat16)
    xT[:KSP] = _bf16(xs.T)
    # dense features ride in the padding rows 416:429 (k-tile 3 rows 32:45)
    xT[KSP:KSP + D] = _bf16(inputs['dense_features'][lo:hi].astype(np.float32).T)
    dom = inputs['domain_ids'][lo:hi].astype(np.int64)
    dom1h = np.zeros((ND, B_LOC), dtype=ml_dtypes.bfloat16)
    for d in range(ND):
        dom1h[d] = (dom == d).astype(np.float32)
    return {'xT': xT, 'dom1h': dom1h}


def build_program(relu_dve=True):
    """relu_dve: move L0b0/L1b0 relu branches to DVE tensor_scalar (max,mult).
    Only valid when b_l0b0/b_l1b0 are all-zero (checked by caller)."""
    nc = bacc.Bacc(trn_type="TRN2", target_bir_lowering=False, debug=False)

    # ---- DRAM I/O ----
    t_xT = nc.dram_tensor('xT', [KPAD, B_LOC], BF16, kind="ExternalInput").ap()
    t_dom1h = nc.dram_tensor('dom1h', [ND, B_LOC], BF16, kind="ExternalInput").ap()
    t_wb16 = nc.dram_tensor('wb16', [128, WB16_COLS], BF16, kind="ExternalInput").ap()
    t_wf32 = nc.dram_tensor('wf32', [128, WF32_COLS], F32, kind="ExternalInput").ap()
    t_out = nc.dram_tensor('out', [B_LOC, OUT], F32, kind="ExternalOutput").ap()

    KT_ROWS = [128, 128, 128, 64]   # xT sbuf k-tiling
    K3 = 128

    with tile.TileContext(nc) as tc:
        with (
            tc.tile_pool(name="wpool", bufs=1) as wpool,
            tc.tile_pool(name="xpool", bufs=4) as xpool,
            tc.tile_pool(name="apool", bufs=2) as apool,
            tc.tile_pool(name="hpool", bufs=2) as hpool,
            tc.tile_pool(name="bcpool", bufs=4) as bcpool,
            tc.tile_pool(name="spool", bufs=4) as spool,
            tc.tile_pool(name="opool", bufs=2) as opool,
            tc.tile_pool(name="ps_mm", bufs=3, space="PSUM") as ps_mm,
            tc.tile_pool(name="ps_smlt", bufs=2, space="PSUM") as ps_smlt,
            tc.tile_pool(name="ps_bc", bufs=3, space="PSUM") as ps_bc,
        ):
            # ---- prologue: resident weights/constants ----
            # two bulk weight loads; everything else is an AP slice of them
            wbT = wpool.tile([128, WB16_COLS], BF16, tag="wb16", name="wb16")
            nc.sync.dma_start(wbT[:], t_wb16)
            wfT = wpool.tile([128, WF32_COLS], F32, tag="wf32", name="wf32")
            nc.sync.dma_start(wfT[:], t_wf32)

            def S16(name):
                off, rows, cols = WB16_LAYOUT[name]
                return wbT[0:rows, off:off + cols]

            def S32(name):
                off, rows, cols = WF32_LAYOUT[name]
                return wfT[0:rows, off:off + cols]

            sWl0 = [S16(f'Wl0_{kt}') for kt in range(4)]
            sGs = [S16(f'Gs{kt}') for kt in range(4)]
            sGq = [S16(f'Gq{kt}') for kt in range(4)]
            sWg = [S16(f'Wg{kt}') for kt in range(4)]
            sSel = S16('sel16')
            sR16 = S16('r16sel')
            sOh3 = S16('oh3')
            sSel12 = S16('sel12')
            sWb1 = [[S16(f'Wb1_{n}{kt}') for kt in range(2)] for n in range(NE)]
            sW10 = [S16(f'W10_{n}') for n in range(NE)]
            sW11 = [[S16(f'W11_{n}{kt}') for kt in range(2)] for n in range(NE)]
            sGb = S32('gbias')
            sB00 = S32('bl0b0')
            sB01 = S32('bl0b1')
            sB10 = S32('bl1b0')
            sB11 = S32('bl1b1')
            sWmix = S32('wmix')
            sId = S16('ident')
            sSelBc = [S16(f'selbc{r}') for r in range(16)]
            sSelBr = [S16(f'selbr{r}') for r in range(4)]

            # per-chunk state carried between phases
            xk = [None] * NCHUNK
            hyb = [None] * NCHUNK
            e0bf = [None] * NCHUNK
            e1bf = [None] * NCHUNK
            e0n = [None] * NCHUNK
            wn = [None] * NCHUNK
            hA = [None] * NCHUNK
            hB = [None] * NCHUNK
            mixed = [None] * NCHUNK
            hC = [None] * NCHUNK
            h2 = [None] * NCHUNK

            def mixed_op_tail(p, out_t, c, bcol, wcol, relu_on_dve, tmp_tag):
                """candidate-mix tail for one [128,NBC] branch psum tile."""
                if c == 0:
                    if relu_on_dve:
                        nc.vector.tensor_scalar(out_t[:], p[:], 0.0, wcol,
                                                ALU.max, ALU.mult)
                    else:
                        nc.scalar.activation(out_t[:], p[:], AF.Relu,
                                             bias=bcol, scale=wcol)
                else:
                    fn = AF.Gelu_apprx_tanh if c == 1 else AF.Tanh
                    tmp = apool.tile([128, NBC], BF16, tag=tmp_tag,
                                     name=f"t{tmp_tag}_{next(uid)}")
                    nc.scalar.activation(tmp[:], p[:], fn, bias=bcol)
                    tw = apool.tile([128, NBC], BF16, tag="tw" + tmp_tag,
                                    name=f"w{tmp_tag}_{next(uid)}")
                    nc.vector.tensor_scalar(tw[:], tmp[:], wcol, None, ALU.mult)
                    nc.vector.tensor_tensor(out_t[:], out_t[:], tw[:], ALU.add)

            import itertools
            uid = itertools.count()

            # ============ P0: loads, squares, fm, gates, softmax prep ============
            def phase0(ch):
                cc = ch * NBC
                xk[ch] = []
                for kt in range(4):
                    rows = 128 if kt == 3 else KT_ROWS[kt]
                    t = xpool.tile([rows, NBC], BF16, tag=f"x{kt}", name=f"x{kt}_{ch}")
                    nc.sync.dma_start(t[0:KT_ROWS[kt], :],
                                      t_xT[kt * 128: kt * 128 + KT_ROWS[kt], cc:cc + NBC])
                    xk[ch].append(t)
                # dense features arrive inside xT rows 416:429; rows 64:128 of
                # the kt3 tile become the fm features below.
                hyb[ch] = xk[ch][3]
                oh = xpool.tile([ND, NBC], BF16, tag="oh", name=f"oh_{ch}")
                nc.sync.dma_start(oh[:], t_dom1h[:, cc:cc + NBC])

                xq = []
                for kt in range(4):
                    t = xpool.tile([KT_ROWS[kt], NBC], BF16, tag=f"xq{kt}", name=f"xq{kt}_{ch}", bufs=2)
                    src = xk[ch][kt][0:KT_ROWS[kt], :]
                    nc.vector.tensor_tensor(t[:], src, src, ALU.mult)
                    xq.append(t)

                sq_ps = ps_smlt.tile([128, NBC], F32, tag="smlt", name=f"sq_{ch}")
                for kt in range(4):
                    nc.tensor.matmul(sq_ps[0:64, :], sGs[kt][:],
                                     xk[ch][kt][0:KT_ROWS[kt], :],
                                     start=(kt == 0), stop=(kt == 3))
                for kt in range(4):
                    nc.tensor.matmul(sq_ps[64:128, :], sGq[kt][:], xq[kt][:],
                                     start=(kt == 0), stop=(kt == 3))
                ssq = spool.tile([64, NBC], F32, tag="ssq", name=f"ssq_{ch}", bufs=2)
                nc.scalar.activation(ssq[:], sq_ps[0:64, :], AF.Square,
                                     scale=float(np.sqrt(0.5)))
                nc.vector.tensor_tensor(hyb[ch][64:128, :], ssq[:], sq_ps[64:128, :],
                                        ALU.subtract)

                g_ps = ps_smlt.tile([44, NBC], F32, tag="smlt", name=f"g_{ch}")
                for kt in range(4):
                    nc.tensor.matmul(g_ps[:], sWg[kt][:],
                                     xk[ch][kt][0:KT_ROWS[kt], :],
                                     start=(kt == 0), stop=(kt == 3))
                gexp = spool.tile([44, NBC], F32, tag="gexp", name=f"gexp_{ch}", bufs=2)
                nc.scalar.activation(gexp[:], g_ps[:], AF.Exp, bias=sGb[:, 0:1])
                e0 = spool.tile([16, NBC], BF16, tag="e0bf", name=f"e0_{ch}", bufs=2)
                nc.vector.tensor_copy(e0[:], gexp[0:16, :])
                e0bf[ch] = e0
                e1 = spool.tile([12, NBC], BF16, tag="e1bf", name=f"e1_{ch}")
                nc.vector.tensor_copy(e1[:], gexp[32:44, :])
                e1bf[ch] = e1
                # layer-0 gate softmax normalize: e0n = e0 * bcast16(1/rowsum)
                s_ps = ps_smlt.tile([4, NBC], F32, tag="smlt", name=f"s0_{ch}")
                nc.tensor.matmul(s_ps[:], sSel[:], e0[:], start=True, stop=True)
                rf = spool.tile([4, NBC], F32, tag="r0f", name=f"r0f_{ch}", bufs=2)
                nc.vector.reciprocal_approx_fast(rf[:], s_ps[:])
                r = spool.tile([4, NBC], BF16, tag="r0", name=f"r0_{ch}", bufs=2)
                nc.vector.tensor_scalar(r[:], rf[:], 1.0, None, ALU.mult)
                r16_ps = ps_smlt.tile([16, NBC], F32, tag="smlt", name=f"r16_{ch}")
                nc.tensor.matmul(r16_ps[:], sR16[:], r[:], start=True, stop=True)
                en = spool.tile([16, NBC], BF16, tag="e0n", name=f"e0n_{ch}")
                nc.vector.tensor_tensor(en[:], e0[:], r16_ps[:], ALU.mult)
                e0n[ch] = en
                # domain gate weights: mask by onehot, expert-sum + select, norm
                ohb_ps = ps_smlt.tile([12, NBC], F32, tag="smlt", name=f"ohb_{ch}")
                nc.tensor.matmul(ohb_ps[:], sOh3[:], oh[:], start=True, stop=True)
                ws12 = spool.tile([12, NBC], BF16, tag="ws", name=f"ws_{ch}", bufs=2)
                nc.vector.tensor_tensor(ws12[:], e1[:], ohb_ps[:], ALU.mult)
                sw_ps = ps_smlt.tile([36, NBC], F32, tag="smlt", name=f"sw_{ch}")
                nc.tensor.matmul(sw_ps[:], sSel12[:], ws12[:], start=True, stop=True)
                rw = spool.tile([4, NBC], F32, tag="rw", name=f"rw_{ch}", bufs=2)
                nc.vector.reciprocal_approx_fast(rw[:], sw_ps[0:4, :])
                wnt = spool.tile([4, NBC], BF16, tag="wn", name=f"wn_{ch}")
                nc.vector.tensor_tensor(wnt[:], sw_ps[32:36, :], rw[:], ALU.mult)
                wn[ch] = wnt

            # ============ P1: L0b0 + mixA -> hA ; L0b1 + mixB -> hB ============
            def phase1(ch):
                hA[ch] = {}
                for n in range(NE):
                    for hh in range(2):
                        hA[ch][(n, hh)] = hpool.tile([128, NBC], BF16, tag=f"hA{n}{hh}",
                                                     name=f"hA{n}{hh}_{ch}")
                    for c in range(NC):
                        for hh in range(2):
                            m = n * 6 + c * 2 + hh
                            p = ps_mm.tile([128, NBC], F32, tag="pmm", name=f"pA{m}_{ch}")
                            for kt in range(3):
                                nc.tensor.matmul(p[:], sWl0[kt][:, m * 128:(m + 1) * 128],
                                                 xk[ch][kt][:], start=(kt == 0), stop=False)
                            nc.tensor.matmul(p[:], sWl0[3][0:K3, m * 128:(m + 1) * 128],
                                             hyb[ch][:], start=False, stop=True)
                            mixed_op_tail(p, hA[ch][(n, hh)], c, sB00[:, m:m + 1],
                                          sWmix[:, n * 3 + c: n * 3 + c + 1],
                                          False, f"A{hh}")
                hB[ch] = {}
                for n in range(NE):
                    hb = hpool.tile([128, NBC], BF16, tag=f"hB{n}", name=f"hB{n}_{ch}")
                    hB[ch][n] = hb
                    for c in range(NC):
                        p = ps_mm.tile([128, NBC], F32, tag="pmm", name=f"pB{n}{c}_{ch}")
                        for kt in range(2):
                            nc.tensor.matmul(p[:], sWb1[n][kt][:, c * 128:(c + 1) * 128],
                                             hA[ch][(n, kt)][:], start=(kt == 0), stop=(kt == 1))
                        m = n * 3 + c
                        mixed_op_tail(p, hb, c, sB01[:, m:m + 1],
                                      sWmix[:, 12 + m: 12 + m + 1], False, "B")

            # ============ P2: expert mixing 0 ============
            def phase2(ch):
                # broadcast rows of the NORMALIZED gate weights; no rp multiply
                mixed[ch] = {}
                for n in range(NE):
                    bcb = []
                    for e in range(NE):
                        bp = ps_bc.tile([128, NBC], F32, tag="bcp", name=f"bcp{n}{e}_{ch}")
                        nc.tensor.matmul(bp[:], sSelBc[e * 4 + n][:], e0n[ch][:],
                                         start=True, stop=True)
                        if e % 2 == 0:
                            # ACT copies psum->sbuf bf16; DVE multiplies from sbuf
                            bb = bcpool.tile([128, NBC], BF16, tag="bcb",
                                             name=f"bcb{n}{e}_{ch}")
                            nc.scalar.copy(bb[:], bp[:])
                            bcb.append(bb)
                        else:
                            bcb.append(bp)
                    mx = hpool.tile([128, NBC], BF16, tag=f"mix{n}", name=f"mix{n}_{ch}")
                    t0 = bcpool.tile([128, NBC], BF16, tag="mixacc", name=f"acc{n}_{ch}")
                    nc.vector.tensor_tensor(t0[:], hB[ch][0][:], bcb[0][:], ALU.mult)
                    for e in range(1, NE):
                        t2 = bcpool.tile([128, NBC], BF16, tag="mixt", name=f"mixt{n}{e}_{ch}")
                        nc.vector.tensor_tensor(t2[:], hB[ch][e][:], bcb[e][:], ALU.mult)
                        if e < NE - 1:
                            nc.vector.tensor_tensor(t0[:], t0[:], t2[:], ALU.add)
                        else:
                            nc.vector.tensor_tensor(mx[:], t0[:], t2[:], ALU.add)
                    mixed[ch][n] = mx

            # ============ P3: L1b0 + mixC -> hC ; L1b1 + mixD -> h2 ============
            def phase3(ch):
                hC[ch] = {}
                for n in range(NE):
                    for hh in range(2):
                        hC[ch][(n, hh)] = hpool.tile([128, NBC], BF16, tag=f"hC{n}{hh}",
                                                     name=f"hC{n}{hh}_{ch}")
                    for c in range(NC):
                        for hh in range(2):
                            m = n * 6 + c * 2 + hh
                            mt = c * 2 + hh
                            p = ps_mm.tile([128, NBC], F32, tag="pmm", name=f"pC{m}_{ch}")
                            nc.tensor.matmul(p[:], sW10[n][:, mt * 128:(mt + 1) * 128],
                                             mixed[ch][n][:], start=True, stop=True)
                            mixed_op_tail(p, hC[ch][(n, hh)], c, sB10[:, m:m + 1],
                                          sWmix[:, 24 + n * 3 + c: 24 + n * 3 + c + 1],
                                          False, f"C{hh}")
                h2[ch] = {}
                for n in range(NE):
                    hb = hpool.tile([128, NBC], BF16, tag=f"h2{n}", name=f"h2{n}_{ch}")
                    h2[ch][n] = hb
                    for c in range(NC):
                        p = ps_mm.tile([128, NBC], F32, tag="pmm", name=f"pD{n}{c}_{ch}")
                        for kt in range(2):
                            nc.tensor.matmul(p[:], sW11[n][kt][:, c * 128:(c + 1) * 128],
                                             hC[ch][(n, kt)][:], start=(kt == 0), stop=(kt == 1))
                        m = n * 3 + c
                        mixed_op_tail(p, hb, c, sB11[:, m:m + 1],
                                      sWmix[:, 36 + m: 36 + m + 1], False, "Dx")

            # ============ P4: domain mix (expert-major) + transpose + out ============
            def phase4(ch):
                cc = ch * NBC
                em = opool.tile([128, NBC], BF16, tag="em", name=f"em_{ch}", bufs=1)
                wb = []
                for e in range(2):
                    bp = ps_bc.tile([128, NBC], F32, tag="bcp", name=f"wb{e}_{ch}")
                    nc.tensor.matmul(bp[:], sSelBr[e][:], wn[ch][:],
                                     start=True, stop=True)
                    wb.append(bp)
                nc.vector.tensor_tensor(em[:], h2[ch][0][:], wb[0][:], ALU.mult)
                for e in range(1, NE):
                    if e + 1 < NE:
                        bp = ps_bc.tile([128, NBC], F32, tag="bcp",
                                        name=f"wb{e + 1}_{ch}")
                        nc.tensor.matmul(bp[:], sSelBr[e + 1][:], wn[ch][:],
                                         start=True, stop=True)
                        wb.append(bp)
                    t2 = opool.tile([128, NBC], BF16, tag="emt", name=f"emt{e}_{ch}", bufs=1)
                    nc.vector.tensor_tensor(t2[:], h2[ch][e][:], wb[e][:], ALU.mult)
                    nc.vector.tensor_tensor(em[:], em[:], t2[:], ALU.add)
                tp = ps_smlt.tile([128, NBC], BF16, tag="smlt", name=f"otp_{ch}")
                for bt in range(4):
                    nc.tensor.transpose(tp[:, bt * 128:(bt + 1) * 128],
                                        em[:, bt * 128:(bt + 1) * 128], sId[:])
                ote = opool.tile([128, NBC], F32, tag="ote", name=f"ote_{ch}", bufs=1)
                nc.scalar.copy(ote[:], tp[:])
                for bt in range(4):
                    nc.sync.dma_start(t_out[cc + bt * 128: cc + (bt + 1) * 128, :],
                                      ote[:, bt * 128:(bt + 1) * 128])

            # ---- emission schedule: P0 all, then rounds with one-chunk lag ----
            for ch in range(NCHUNK):
                phase0(ch)
            for ch in range(NCHUNK):
                phase1(ch)
                if ch > 0:
                    phase2(ch - 1)
                    phase3(ch - 1)
                    phase4(ch - 1)
            phase2(NCHUNK - 1)
            phase3(NCHUNK - 1)
            phase4(NCHUNK - 1)
    nc.compile()
    return nc


_CACHE = {}


def kernel(**inputs):
    shared = prep_shared(inputs)
    in_maps = []
    for r in range(N_CORES):
        m = dict(shared)
        m.update(prep_core(inputs, r))
        in_maps.append(m)
    relu_dve = (np.abs(inputs['b_l0b0']).max() == 0.0
                and np.abs(inputs['b_l1b0']).max() == 0.0)
    key = ('nc', bool(relu_dve))
    if key not in _CACHE:
        _CACHE[key] = build_program(relu_dve=relu_dve)
        _CACHE['nc'] = _CACHE[key]
    nc = _CACHE[key]
    res = run_bass_kernel_spmd(nc, in_maps, core_ids=list(range(N_CORES)))
    out = np.concatenate([res.results[r]['out'] for r in range(N_CORES)], axis=0)
    return out.astype(np.float32)



# revision 101
# speedup vs baseline: 1.0157x; 1.0020x over previous
"""AutoMTLSuperNet (moe_routing) Trainium2 kernel.

Strategy: batch data-parallel over 8 NeuronCores (2048 samples each, params
replicated). On-chip layout is output-channel-major ([oc, batch]) so every
layer's matmul output is directly the next layer's moving operand. All
matmuls run in bf16 with f32 PSUM accumulation; batch is processed in
chunks of 512 columns.

Host-side prep is parameter-only + input layout: transposes, padding,
folding sigmoid(feat_alpha) into the L0 weight rows, FM contraction
matrices, candidate-softmax weights, merged gate biases. Dense features
ride in the zero-padding rows of xT so the hybrid k-tile loads in one DMA.

Engine balance (measured per-op costs on HW): all candidate-relu branches
run on ACT (relu(w*p + w*b) via the scale port); gate softmaxes are
normalized up front (reciprocal_approx_fast + row-broadcast matmul) so the
expert mix needs no per-n reciprocal multiply; half the expert-mix
broadcasts are consumed directly from f32 PSUM by DVE, half via ACT
psum->bf16 copies; the final domain mix is computed expert-major (PE row
broadcasts of the domain-selected, normalized gate weights) and transposed
once at the end.
"""

import numpy as np
import ml_dtypes

import concourse.bass as bass
import concourse.bacc as bacc
import concourse.mybir as mybir
import concourse.tile as tile
from concourse.bass_utils import run_bass_kernel_spmd

# ---- problem dims (hardcoded per contract) ----
B, F, E, D = 16384, 26, 16, 13
NE, ND, NC = 4, 3, 3
GIN = E * (F + 1) + D            # 445
H, OUT = 256, 128
N_CORES = 8
B_LOC = B // N_CORES             # 2048
NBC = 512                        # batch columns per chunk
NCHUNK = B_LOC // NBC            # 4
KSP = F * E                      # 416 flattened sparse dim
KPAD = 448                       # padded to 4 x (128,128,128,64)
BF16 = mybir.dt.bfloat16
F32 = mybir.dt.float32

AF = mybir.ActivationFunctionType
ALU = mybir.AluOpType


# ---- packed weight bundle layouts (name -> (col_off, rows, cols)) ----
def _mk_layout(blocks):
    out, cur = {}, 0
    for name, rows, cols in blocks:
        out[name] = (cur, rows, cols)
        cur += cols
    return out, cur

KT_ROWS = [128, 128, 128, 64]
_WB16_BLOCKS = (
    [(f'Wl0_{k}', 128, 3072) for k in range(4)]
    + [(f'Gs{k}', KT_ROWS[k], 64) for k in range(4)]
    + [(f'Gq{k}', KT_ROWS[k], 64) for k in range(4)]
    + [(f'Wg{k}', KT_ROWS[k], 44) for k in range(4)]
    + [('sel16', 16, 4), ('r16sel', 4, 16), ('oh3', 3, 12), ('sel12', 12, 36)]
    + [(f'selbc{r}', 16, 128) for r in range(16)]
    + [(f'selbr{r}', 4, 128) for r in range(4)]
    + [(f'Wb1_{n}{k}', 128, 384) for n in range(4) for k in range(2)]
    + [(f'W10_{n}', 128, 768) for n in range(4)]
    + [(f'W11_{n}{k}', 128, 384) for n in range(4) for k in range(2)]
    + [('ident', 128, 128)]
)
_WF32_BLOCKS = [('gbias', 44, 1), ('bl0b0', 128, 24), ('bl0b1', 128, 12),
                ('bl1b0', 128, 24), ('bl1b1', 128, 12), ('wmix', 128, 48)]
WB16_LAYOUT, WB16_COLS = _mk_layout(_WB16_BLOCKS)
WF32_LAYOUT, WF32_COLS = _mk_layout(_WF32_BLOCKS)


def _bf16(x):
    return np.asarray(x, dtype=ml_dtypes.bfloat16)


def _softmax_np(a):
    a = np.asarray(a, dtype=np.float64)
    m = a.max(axis=-1, keepdims=True)
    e = np.exp(a - m)
    return (e / e.sum(axis=-1, keepdims=True)).astype(np.float32)


def prep_shared(inputs):
    """Host prep of all parameter tensors (input-layout + parameter-only math)."""
    f32 = np.float32
    gate_w = 1.0 / (1.0 + np.exp(-inputs['feat_alpha'].astype(np.float64)))  # [NE,F]
    gate_w = gate_w.astype(f32)

    W_l0b0 = inputs['W_l0b0'].astype(f32)   # [NE,NC,GIN,H]
    W_l0b1 = inputs['W_l0b1'].astype(f32)   # [NE,NC,H,OUT]
    W_l1b0 = inputs['W_l1b0'].astype(f32)   # [NE,NC,OUT,H]
    W_l1b1 = inputs['W_l1b1'].astype(f32)   # [NE,NC,H,OUT]

    # candidate softmax weights per mixed-op layer: [4][NE,NC]
    wmix_l = [_softmax_np(inputs[k]) for k in ('a_l0b0', 'a_l0b1', 'a_l1b0', 'a_l1b1')]

    # ---- Wl0: lhsT ktiles [4,128, 3072]; col = n*768 + c*256 + h ----
    Wl0 = np.zeros((4, 128, NE * NC * H), dtype=f32)
    # sparse rows (g-folded): global row i = fe for fe in 0..415
    Wsp = np.zeros((KSP, NE, NC, H), dtype=f32)
    for n in range(NE):
        gvec = np.repeat(gate_w[n], E)                      # [416]
        Wsp[:, n] = W_l0b0[n, :, :KSP, :].transpose(1, 0, 2) * gvec[:, None, None]
    Wsp = Wsp.reshape(KSP, NE * NC * H)
    for kt in range(3):
        Wl0[kt, :, :] = Wsp[kt * 128:(kt + 1) * 128]
    # kt3 layout: [0:32]=sparse rows 384..415, [32:45]=dense, [45:64]=0,
    #             [64:128]=fm rows (64 + n*16 + e)
    Wl0[3, 0:32, :] = Wsp[384:416]
    for d in range(D):
        Wl0[3, 32 + d, :] = W_l0b0[:, :, KSP + E + d, :].reshape(-1)
    for n in range(NE):
        for e in range(E):
            Wl0[3, 64 + n * 16 + e, n * 768:(n + 1) * 768] = \
                W_l0b0[n, :, KSP + e, :].reshape(768)

    # ---- Gs / Gq: [4,128,64]  col = n*16+e ; row = fe (per ktile of xT) ----
    Gs = np.zeros((4, 128, 64), dtype=f32)
    Gq = np.zeros((4, 128, 64), dtype=f32)
    for fe in range(KSP):
        kt, i = divmod(fe, 128)
        f_, e_ = divmod(fe, E)
        for n in range(NE):
            g = gate_w[n, f_]
            Gs[kt, i, n * 16 + e_] = g
            Gq[kt, i, n * 16 + e_] = 0.5 * g * g   # 0.5 pre-folded
    # ---- Wg: [4,128,44]: cols 0-15 g0 (e*4+n), 32-43 g1 (32+d*4+e) ----
    Wg = np.zeros((4, 128, 44), dtype=f32)
    Wg0, Wg1 = inputs['Wg0'].astype(f32), inputs['Wg1'].astype(f32)
    for i in range(KSP):
        kt, r = divmod(i, 128)
        for n in range(NE):
            for e in range(NE):
                Wg[kt, r, e * 4 + n] = Wg0[n, i, e]
        for d in range(ND):
            for e in range(NE):
                Wg[kt, r, 32 + d * 4 + e] = Wg1[d, i, e]
    gbias = np.zeros((44, 1), dtype=f32)
    for n in range(NE):
        for e in range(NE):
            gbias[e * 4 + n, 0] = inputs['bg0'][n, e] + inputs['beta0'][n, e]
    for d in range(ND):
        for e in range(NE):
            gbias[32 + d * 4 + e, 0] = inputs['bg1'][d, e] + inputs['beta1'][d, e]
    # sel16 [16,4]: row e*4+n -> col n
    sel16 = np.zeros((16, 4), dtype=f32)
    for e in range(NE):
        for n in range(NE):
            sel16[e * 4 + n, n] = 1.0
    # r16sel [4,16]: broadcast r0 row n to rows e*4+n
    r16sel = np.zeros((4, 16), dtype=f32)
    for e in range(NE):
        for n in range(NE):
            r16sel[n, e * 4 + n] = 1.0
    # oh3 [3,12]: broadcast domain-onehot row d to rows 4d..4d+3
    oh3 = np.zeros((3, 12), dtype=f32)
    for d in range(ND):
        oh3[d, 4 * d:4 * d + 4] = 1.0
    # sel12 [12,36]: cols 0:4 = all-ones (expert sum); cols 32:36 pick expert e
    sel12 = np.zeros((12, 36), dtype=f32)
    sel12[:, 0:4] = 1.0
    for d in range(ND):
        for e in range(NE):
            sel12[4 * d + e, 32 + e] = 1.0

    # ---- later layer weights ----
    Wb1 = np.zeros((NE, H, NC * OUT), dtype=f32)       # lhsT col = c*128+o
    for n in range(NE):
        Wb1[n] = W_l0b1[n].transpose(1, 0, 2).reshape(H, NC * OUT)
    W10 = np.zeros((NE, OUT, NC * H), dtype=f32)       # col = c*256+h
    for n in range(NE):
        W10[n] = W_l1b0[n].transpose(1, 0, 2).reshape(OUT, NC * H)
    W11 = np.zeros((NE, H, NC * OUT), dtype=f32)
    for n in range(NE):
        W11[n] = W_l1b1[n].transpose(1, 0, 2).reshape(H, NC * OUT)

    # ---- bias column tables (per-partition vectors), w-scaled for relu c=0 ----
    def bias_table(bmat, wl, n_mt):  # bmat [NE,NC,W]; returns [128, NE*NC*n_mt]
        Wd = bmat.shape[-1]
        tbl = np.zeros((128, NE * NC * (Wd // 128)), dtype=f32)
        m = 0
        for n in range(NE):
            for c in range(NC):
                for hh in range(Wd // 128):
                    v = bmat[n, c, hh * 128:(hh + 1) * 128].astype(f32)
                    if c == 0:
                        v = v * wl[n, 0]
                    tbl[:, m] = v
                    m += 1
        return tbl
    bl0b0 = bias_table(inputs['b_l0b0'], wmix_l[0], 2)   # [128,24]
    bl0b1 = bias_table(inputs['b_l0b1'], wmix_l[1], 1)   # [128,12]
    bl1b0 = bias_table(inputs['b_l1b0'], wmix_l[2], 2)   # [128,24]
    bl1b1 = bias_table(inputs['b_l1b1'], wmix_l[3], 1)   # [128,12]

    wmix = np.zeros((128, 48), dtype=f32)
    for li, wl in enumerate(wmix_l):
        for n in range(NE):
            for c in range(NC):
                wmix[:, li * 12 + n * 3 + c] = wl[n, c]

    ident = np.eye(128, dtype=f32)
    ones1 = np.ones((1, 128), dtype=f32)
    # broadcast selectors: selbc[r] = e_r (x) ones128  -> lhsT picks row r of rhs
    selbc = np.zeros((16, 16, 128), dtype=f32)
    for r_ in range(16):
        selbc[r_, r_, :] = 1.0
    selbr = np.zeros((4, 4, 128), dtype=f32)
    for r_ in range(4):
        selbr[r_, r_, :] = 1.0

    # pack everything into two bundles so the device needs just two DMAs
    wb16 = np.zeros((128, WB16_COLS), dtype=ml_dtypes.bfloat16)
    wf32 = np.zeros((128, WF32_COLS), dtype=f32)

    def put16(name, arr):
        off, rows, cols = WB16_LAYOUT[name]
        wb16[0:rows, off:off + cols] = _bf16(arr)

    def put32(name, arr):
        off, rows, cols = WF32_LAYOUT[name]
        wf32[0:rows, off:off + cols] = arr

    for k in range(4):
        put16(f'Wl0_{k}', Wl0[k])
        put16(f'Gs{k}', Gs[k][:KT_ROWS[k]])
        put16(f'Gq{k}', Gq[k][:KT_ROWS[k]])
        put16(f'Wg{k}', Wg[k][:KT_ROWS[k]])
    put16('sel16', sel16); put16('r16sel', r16sel)
    put16('oh3', oh3); put16('sel12', sel12)
    for r_ in range(16):
        put16(f'selbc{r_}', selbc[r_])
    for r_ in range(4):
        put16(f'selbr{r_}', selbr[r_])
    for n in range(NE):
        for k in range(2):
            put16(f'Wb1_{n}{k}', Wb1[n][k * 128:(k + 1) * 128, :])
            put16(f'W11_{n}{k}', W11[n][k * 128:(k + 1) * 128, :])
        put16(f'W10_{n}', W10[n])
    put16('ident', ident)
    put32('gbias', gbias); put32('bl0b0', bl0b0); put32('bl0b1', bl0b1)
    put32('bl1b0', bl1b0); put32('bl1b1', bl1b1); put32('wmix', wmix)
    return {'wb16': wb16, 'wf32': wf32}


def prep_core(inputs, r):
    """Per-core input shards (layout only)."""
    lo, hi = r * B_LOC, (r + 1) * B_LOC
    xs = inputs['sparse_embs'][lo:hi].reshape(B_LOC, KSP)      # [2048,416] f32
    xT = np.zeros((KPAD, B_LOC), dtype=ml_dtypes.bfloat16)
    xT[:KSP] = _bf16(xs.T)
    # dense features ride in the padding rows 416:429 (k-tile 3 rows 32:45)
    xT[KSP:KSP + D] = _bf16(inputs['dense_features'][lo:hi].astype(np.float32).T)
    dom = inputs['domain_ids'][lo:hi].astype(np.int64)
    dom1h = np.zeros((ND, B_LOC), dtype=ml_dtypes.bfloat16)
    for d in range(ND):
        dom1h[d] = (dom == d).astype(np.float32)
    return {'xT': xT, 'dom1h': dom1h}


def build_program(relu_dve=True):
    """relu_dve: move L0b0/L1b0 relu branches to DVE tensor_scalar (max,mult).
    Only valid when b_l0b0/b_l1b0 are all-zero (checked by caller)."""
    nc = bacc.Bacc(trn_type="TRN2", target_bir_lowering=False, debug=False)

    # ---- DRAM I/O ----
    t_xT = nc.dram_tensor('xT', [KPAD, B_LOC], BF16, kind="ExternalInput").ap()
    t_dom1h = nc.dram_tensor('dom1h', [ND, B_LOC], BF16, kind="ExternalInput").ap()
    t_wb16 = nc.dram_tensor('wb16', [128, WB16_COLS], BF16, kind="ExternalInput").ap()
    t_wf32 = nc.dram_tensor('wf32', [128, WF32_COLS], F32, kind="ExternalInput").ap()
    t_out = nc.dram_tensor('out', [B_LOC, OUT], F32, kind="ExternalOutput").ap()

    KT_ROWS = [128, 128, 128, 64]   # xT sbuf k-tiling
    K3 = 128

    with tile.TileContext(nc) as tc:
        with (
            tc.tile_pool(name="wpool", bufs=1) as wpool,
            tc.tile_pool(name="xpool", bufs=4) as xpool,
            tc.tile_pool(name="apool", bufs=2) as apool,
            tc.tile_pool(name="hpool", bufs=2) as hpool,
            tc.tile_pool(name="bcpool", bufs=4) as bcpool,
            tc.tile_pool(name="spool", bufs=4) as spool,
            tc.tile_pool(name="opool", bufs=2) as opool,
            tc.tile_pool(name="ps_mm", bufs=4, space="PSUM") as ps_mm,
            tc.tile_pool(name="ps_smlt", bufs=1, space="PSUM") as ps_smlt,
            tc.tile_pool(name="ps_bc", bufs=3, space="PSUM") as ps_bc,
        ):
            # ---- prologue: resident weights/constants ----
            # two bulk weight loads; everything else is an AP slice of them
            wbT = wpool.tile([128, WB16_COLS], BF16, tag="wb16", name="wb16")
            nc.sync.dma_start(wbT[:], t_wb16)
            wfT = wpool.tile([128, WF32_COLS], F32, tag="wf32", name="wf32")
            nc.sync.dma_start(wfT[:], t_wf32)

            def S16(name):
                off, rows, cols = WB16_LAYOUT[name]
                return wbT[0:rows, off:off + cols]

            def S32(name):
                off, rows, cols = WF32_LAYOUT[name]
                return wfT[0:rows, off:off + cols]

            sWl0 = [S16(f'Wl0_{kt}') for kt in range(4)]
            sGs = [S16(f'Gs{kt}') for kt in range(4)]
            sGq = [S16(f'Gq{kt}') for kt in range(4)]
            sWg = [S16(f'Wg{kt}') for kt in range(4)]
            sSel = S16('sel16')
            sR16 = S16('r16sel')
            sOh3 = S16('oh3')
            sSel12 = S16('sel12')
            sWb1 = [[S16(f'Wb1_{n}{kt}') for kt in range(2)] for n in range(NE)]
            sW10 = [S16(f'W10_{n}') for n in range(NE)]
            sW11 = [[S16(f'W11_{n}{kt}') for kt in range(2)] for n in range(NE)]
            sGb = S32('gbias')
            sB00 = S32('bl0b0')
            sB01 = S32('bl0b1')
            sB10 = S32('bl1b0')
            sB11 = S32('bl1b1')
            sWmix = S32('wmix')
            sId = S16('ident')
            sSelBc = [S16(f'selbc{r}') for r in range(16)]
            sSelBr = [S16(f'selbr{r}') for r in range(4)]

            # per-chunk state carried between phases
            xk = [None] * NCHUNK
            hyb = [None] * NCHUNK
            e0bf = [None] * NCHUNK
            e1bf = [None] * NCHUNK
            e0n = [None] * NCHUNK
            wn = [None] * NCHUNK
            hA = [None] * NCHUNK
            hB = [None] * NCHUNK
            mixed = [None] * NCHUNK
            hC = [None] * NCHUNK
            h2 = [None] * NCHUNK

            def mixed_op_tail(p, out_t, c, bcol, wcol, relu_on_dve, tmp_tag):
                """candidate-mix tail for one [128,NBC] branch psum tile."""
                if c == 0:
                    if relu_on_dve:
                        nc.vector.tensor_scalar(out_t[:], p[:], 0.0, wcol,
                                                ALU.max, ALU.mult)
                    else:
                        nc.scalar.activation(out_t[:], p[:], AF.Relu,
                                             bias=bcol, scale=wcol)
                else:
                    fn = AF.Gelu_apprx_tanh if c == 1 else AF.Tanh
                    tmp = apool.tile([128, NBC], BF16, tag=tmp_tag,
                                     name=f"t{tmp_tag}_{next(uid)}")
                    nc.scalar.activation(tmp[:], p[:], fn, bias=bcol)
                    tw = apool.tile([128, NBC], BF16, tag="tw" + tmp_tag,
                                    name=f"w{tmp_tag}_{next(uid)}")
                    nc.vector.tensor_scalar(tw[:], tmp[:], wcol, None, ALU.mult)
                    nc.vector.tensor_tensor(out_t[:], out_t[:], tw[:], ALU.add)

            import itertools
            uid = itertools.count()

            # ============ P0: loads, squares, fm, gates, softmax prep ============
            def phase0(ch):
                cc = ch * NBC
                xk[ch] = []
                for kt in range(4):
                    rows = 128 if kt == 3 else KT_ROWS[kt]
                    t = xpool.tile([rows, NBC], BF16, tag=f"x{kt}", name=f"x{kt}_{ch}")
                    nc.sync.dma_start(t[0:KT_ROWS[kt], :],
                                      t_xT[kt * 128: kt * 128 + KT_ROWS[kt], cc:cc + NBC])
                    xk[ch].append(t)
                # dense features arrive inside xT rows 416:429; rows 64:128 of
                # the kt3 tile become the fm features below.
                hyb[ch] = xk[ch][3]
                oh = xpool.tile([ND, NBC], BF16, tag="oh", name=f"oh_{ch}")
                nc.sync.dma_start(oh[:], t_dom1h[:, cc:cc + NBC])

                xq = []
                for kt in range(4):
                    t = xpool.tile([KT_ROWS[kt], NBC], BF16, tag=f"xq{kt}", name=f"xq{kt}_{ch}", bufs=2)
                    src = xk[ch][kt][0:KT_ROWS[kt], :]
                    nc.vector.tensor_tensor(t[:], src, src, ALU.mult)
                    xq.append(t)

                sq_ps = ps_smlt.tile([128, NBC], F32, tag="smlt", name=f"sq_{ch}")
                for kt in range(4):
                    nc.tensor.matmul(sq_ps[0:64, :], sGs[kt][:],
                                     xk[ch][kt][0:KT_ROWS[kt], :],
                                     start=(kt == 0), stop=(kt == 3))
                for kt in range(4):
                    nc.tensor.matmul(sq_ps[64:128, :], sGq[kt][:], xq[kt][:],
                                     start=(kt == 0), stop=(kt == 3))
                ssq = spool.tile([64, NBC], F32, tag="ssq", name=f"ssq_{ch}", bufs=2)
                nc.scalar.activation(ssq[:], sq_ps[0:64, :], AF.Square,
                                     scale=float(np.sqrt(0.5)))
                nc.vector.tensor_tensor(hyb[ch][64:128, :], ssq[:], sq_ps[64:128, :],
                                        ALU.subtract)

                g_ps = ps_smlt.tile([44, NBC], F32, tag="smlt", name=f"g_{ch}")
                for kt in range(4):
                    nc.tensor.matmul(g_ps[:], sWg[kt][:],
                                     xk[ch][kt][0:KT_ROWS[kt], :],
                                     start=(kt == 0), stop=(kt == 3))
                gexp = spool.tile([44, NBC], F32, tag="gexp", name=f"gexp_{ch}", bufs=2)
                nc.scalar.activation(gexp[:], g_ps[:], AF.Exp, bias=sGb[:, 0:1])
                e0 = spool.tile([16, NBC], BF16, tag="e0bf", name=f"e0_{ch}", bufs=2)
                nc.vector.tensor_copy(e0[:], gexp[0:16, :])
                e0bf[ch] = e0
                e1 = spool.tile([12, NBC], BF16, tag="e1bf", name=f"e1_{ch}")
                nc.vector.tensor_copy(e1[:], gexp[32:44, :])
                e1bf[ch] = e1
                # layer-0 gate softmax normalize: e0n = e0 * bcast16(1/rowsum)
                s_ps = ps_smlt.tile([4, NBC], F32, tag="smlt", name=f"s0_{ch}")
                nc.tensor.matmul(s_ps[:], sSel[:], e0[:], start=True, stop=True)
                rf = spool.tile([4, NBC], F32, tag="r0f", name=f"r0f_{ch}", bufs=2)
                nc.vector.reciprocal_approx_fast(rf[:], s_ps[:])
                r = spool.tile([4, NBC], BF16, tag="r0", name=f"r0_{ch}", bufs=2)
                nc.vector.tensor_scalar(r[:], rf[:], 1.0, None, ALU.mult)
                r16_ps = ps_smlt.tile([16, NBC], F32, tag="smlt", name=f"r16_{ch}")
                nc.tensor.matmul(r16_ps[:], sR16[:], r[:], start=True, stop=True)
                en = spool.tile([16, NBC], BF16, tag="e0n", name=f"e0n_{ch}")
                nc.vector.tensor_tensor(en[:], e0[:], r16_ps[:], ALU.mult)
                e0n[ch] = en
                # domain gate weights: mask by onehot, expert-sum + select, norm
                ohb_ps = ps_smlt.tile([12, NBC], F32, tag="smlt", name=f"ohb_{ch}")
                nc.tensor.matmul(ohb_ps[:], sOh3[:], oh[:], start=True, stop=True)
                ws12 = spool.tile([12, NBC], BF16, tag="ws", name=f"ws_{ch}", bufs=2)
                nc.vector.tensor_tensor(ws12[:], e1[:], ohb_ps[:], ALU.mult)
                sw_ps = ps_smlt.tile([36, NBC], F32, tag="smlt", name=f"sw_{ch}")
                nc.tensor.matmul(sw_ps[:], sSel12[:], ws12[:], start=True, stop=True)
                rw = spool.tile([4, NBC], F32, tag="rw", name=f"rw_{ch}", bufs=2)
                nc.vector.reciprocal_approx_fast(rw[:], sw_ps[0:4, :])
                wnt = spool.tile([4, NBC], BF16, tag="wn", name=f"wn_{ch}")
                nc.vector.tensor_tensor(wnt[:], sw_ps[32:36, :], rw[:], ALU.mult)
                wn[ch] = wnt

            # ============ P1: L0b0 + mixA -> hA ; L0b1 + mixB -> hB ============
            def phase1(ch):
                hA[ch] = {}
                for n in range(NE):
                    for hh in range(2):
                        hA[ch][(n, hh)] = hpool.tile([128, NBC], BF16, tag=f"hA{n}{hh}",
                                                     name=f"hA{n}{hh}_{ch}")
                    for c in range(NC):
                        for hh in range(2):
                            m = n * 6 + c * 2 + hh
                            p = ps_mm.tile([128, NBC], F32, tag="pmm", name=f"pA{m}_{ch}")
                            for kt in range(3):
                                nc.tensor.matmul(p[:], sWl0[kt][:, m * 128:(m + 1) * 128],
                                                 xk[ch][kt][:], start=(kt == 0), stop=False)
                            nc.tensor.matmul(p[:], sWl0[3][0:K3, m * 128:(m + 1) * 128],
                                             hyb[ch][:], start=False, stop=True)
                            mixed_op_tail(p, hA[ch][(n, hh)], c, sB00[:, m:m + 1],
                                          sWmix[:, n * 3 + c: n * 3 + c + 1],
                                          False, f"A{hh}")
                hB[ch] = {}
                for n in range(NE):
                    hb = hpool.tile([128, NBC], BF16, tag=f"hB{n}", name=f"hB{n}_{ch}")
                    hB[ch][n] = hb
                    for c in range(NC):
                        p = ps_mm.tile([128, NBC], F32, tag="pmm", name=f"pB{n}{c}_{ch}")
                        for kt in range(2):
                            nc.tensor.matmul(p[:], sWb1[n][kt][:, c * 128:(c + 1) * 128],
                                             hA[ch][(n, kt)][:], start=(kt == 0), stop=(kt == 1))
                        m = n * 3 + c
                        mixed_op_tail(p, hb, c, sB01[:, m:m + 1],
                                      sWmix[:, 12 + m: 12 + m + 1], False, "B")

            # ============ P2: expert mixing 0 ============
            def phase2(ch):
                # broadcast rows of the NORMALIZED gate weights; no rp multiply
                mixed[ch] = {}
                for n in range(NE):
                    bcb = []
                    for e in range(NE):
                        bp = ps_bc.tile([128, NBC], F32, tag="bcp", name=f"bcp{n}{e}_{ch}")
                        nc.tensor.matmul(bp[:], sSelBc[e * 4 + n][:], e0n[ch][:],
                                         start=True, stop=True)
                        if e % 2 == 0:
                            # ACT copies psum->sbuf bf16; DVE multiplies from sbuf
                            bb = bcpool.tile([128, NBC], BF16, tag="bcb",
                                             name=f"bcb{n}{e}_{ch}")
                            nc.scalar.copy(bb[:], bp[:])
                            bcb.append(bb)
                        else:
                            bcb.append(bp)
                    mx = hpool.tile([128, NBC], BF16, tag=f"mix{n}", name=f"mix{n}_{ch}")
                    t0 = bcpool.tile([128, NBC], BF16, tag="mixacc", name=f"acc{n}_{ch}")
                    nc.vector.tensor_tensor(t0[:], hB[ch][0][:], bcb[0][:], ALU.mult)
                    for e in range(1, NE):
                        t2 = bcpool.tile([128, NBC], BF16, tag="mixt", name=f"mixt{n}{e}_{ch}")
                        nc.vector.tensor_tensor(t2[:], hB[ch][e][:], bcb[e][:], ALU.mult)
                        if e < NE - 1:
                            nc.vector.tensor_tensor(t0[:], t0[:], t2[:], ALU.add)
                        else:
                            nc.vector.tensor_tensor(mx[:], t0[:], t2[:], ALU.add)
                    mixed[ch][n] = mx

            # ============ P3: L1b0 + mixC -> hC ; L1b1 + mixD -> h2 ============
            def phase3(ch):
                hC[ch] = {}
                for n in range(NE):
                    for hh in range(2):
                        hC[ch][(n, hh)] = hpool.tile([128, NBC], BF16, tag=f"hC{n}{hh}",
                                                     name=f"hC{n}{hh}_{ch}")
                    for c in range(NC):
                        for hh in range(2):
                            m = n * 6 + c * 2 + hh
                            mt = c * 2 + hh
                            p = ps_mm.tile([128, NBC], F32, tag="pmm", name=f"pC{m}_{ch}")
                            nc.tensor.matmul(p[:], sW10[n][:, mt * 128:(mt + 1) * 128],
                                             mixed[ch][n][:], start=True, stop=True)
                            mixed_op_tail(p, hC[ch][(n, hh)], c, sB10[:, m:m + 1],
                                          sWmix[:, 24 + n * 3 + c: 24 + n * 3 + c + 1],
                                          False, f"C{hh}")
                h2[ch] = {}
                for n in range(NE):
                    hb = hpool.tile([128, NBC], BF16, tag=f"h2{n}", name=f"h2{n}_{ch}")
                    h2[ch][n] = hb
                    for c in range(NC):
                        p = ps_mm.tile([128, NBC], F32, tag="pmm", name=f"pD{n}{c}_{ch}")
                        for kt in range(2):
                            nc.tensor.matmul(p[:], sW11[n][kt][:, c * 128:(c + 1) * 128],
                                             hC[ch][(n, kt)][:], start=(kt == 0), stop=(kt == 1))
                        m = n * 3 + c
                        mixed_op_tail(p, hb, c, sB11[:, m:m + 1],
                                      sWmix[:, 36 + m: 36 + m + 1], False, "Dx")

            # ============ P4: domain mix (expert-major) + transpose + out ============
            def phase4(ch):
                cc = ch * NBC
                em = opool.tile([128, NBC], BF16, tag="em", name=f"em_{ch}", bufs=1)
                wb = []
                for e in range(2):
                    bp = ps_bc.tile([128, NBC], F32, tag="bcp", name=f"wb{e}_{ch}")
                    nc.tensor.matmul(bp[:], sSelBr[e][:], wn[ch][:],
                                     start=True, stop=True)
                    wb.append(bp)
                nc.vector.tensor_tensor(em[:], h2[ch][0][:], wb[0][:], ALU.mult)
                for e in range(1, NE):
                    if e + 1 < NE:
                        bp = ps_bc.tile([128, NBC], F32, tag="bcp",
                                        name=f"wb{e + 1}_{ch}")
                        nc.tensor.matmul(bp[:], sSelBr[e + 1][:], wn[ch][:],
                                         start=True, stop=True)
                        wb.append(bp)
                    t2 = opool.tile([128, NBC], BF16, tag="emt", name=f"emt{e}_{ch}", bufs=1)
                    nc.vector.tensor_tensor(t2[:], h2[ch][e][:], wb[e][:], ALU.mult)
                    nc.vector.tensor_tensor(em[:], em[:], t2[:], ALU.add)
                tp = ps_smlt.tile([128, NBC], BF16, tag="smlt", name=f"otp_{ch}")
                for bt in range(4):
                    nc.tensor.transpose(tp[:, bt * 128:(bt + 1) * 128],
                                        em[:, bt * 128:(bt + 1) * 128], sId[:])
                ote = opool.tile([128, NBC], F32, tag="ote", name=f"ote_{ch}", bufs=1)
                nc.scalar.copy(ote[:], tp[:])
                for bt in range(4):
                    nc.sync.dma_start(t_out[cc + bt * 128: cc + (bt + 1) * 128, :],
                                      ote[:, bt * 128:(bt + 1) * 128])

            # ---- emission schedule: P0 all, then rounds with one-chunk lag ----
            for ch in range(NCHUNK):
                phase0(ch)
            for ch in range(NCHUNK):
                phase1(ch)
                if ch > 0:
                    phase2(ch - 1)
                    phase3(ch - 1)
                    phase4(ch - 1)
            phase2(NCHUNK - 1)
            phase3(NCHUNK - 1)
            phase4(NCHUNK - 1)
    nc.compile()
    return nc


_CACHE = {}


def kernel(**inputs):
    shared = prep_shared(inputs)
    in_maps = []
    for r in range(N_CORES):
        m = dict(shared)
        m.update(prep_core(inputs, r))
        in_maps.append(m)
    relu_dve = (np.abs(inputs['b_l0b0']).max() == 0.0
                and np.abs(inputs['b_l1b0']).max() == 0.0)
    key = ('nc', bool(relu_dve))
    if key not in _CACHE:
        _CACHE[key] = build_program(relu_dve=relu_dve)
        _CACHE['nc'] = _CACHE[key]
    nc = _CACHE[key]
    res = run_bass_kernel_spmd(nc, in_maps, core_ids=list(range(N_CORES)))
    out = np.concatenate([res.results[r]['out'] for r in range(N_CORES)], axis=0)
    return out.astype(np.float32)

